# revision 3
# baseline (speedup 1.0000x reference)
"""VQ codebook-lookup kernel for 8 Trainium2 NeuronCores.

Computes z_q = z_e + sg(weight[argmin_k ||z - w_k||^2] - z_e) for
z_e [32,32,32,256], weight [1024, 256] f32, data-parallel over the
flattened token dim (4096 tokens per core, codebook replicated).

The reference evaluates distances in fp32 as
    dist = fl(fl(S1_t + S2_k) - fl(2*(z @ w^T)))
with S1_t = |z_t|^2 summed in XLA-CPU's block8x32 order.  Because
S1 ~ 256 while the argmin-relevant variation is ~1e-2, dist is
quantized to S1's ulp (~3e-5) and ties are common; this kernel
replicates the reference's fp32 rounding structure bitwise so the
selected indices match exactly:

  - S1 via ACT Square + DVE two-level reduce [128,8,32] -> [128,8] -> 1
    (bitwise-verified == XLA's block8-sequential order on HW).
  - T1 = fl(S2_full + S1) on ScalarE (Identity w/ per-partition bias).
  - M2 = (2z) @ w^T on the PE in fp32 (order-insensitive: verified 0
    decision flips across accumulation orders on the fixed inputs).
  - Y = fl(M2 - T1) = -dist on VectorE; argmax with first-index tie
    break via InstMax8 + InstMaxIndex (matches jnp.argmin on ties).
  - Indirect-DMA gather of weight rows; straight-through estimator
    out = fl(z + fl(w[idx] - z)) on GpSimd (bit-identical to the
    reference output).
"""

import numpy as np

N_CORES = 8
D = 256
K = 1024
N_TOTAL = 32 * 32 * 32
N_LOC = N_TOTAL // N_CORES  # 4096
P = 128
N_TILES = N_LOC // P  # 32

_CACHE = {}


def _build(n_tiles: int = N_TILES):
    import concourse.bass as bass
    import concourse.mybir as mybir
    import concourse.tile as tile
    from concourse import bacc
    from concourse.masks import make_identity

    fp32 = mybir.dt.float32
    u32 = mybir.dt.uint32
    alu = mybir.AluOpType
    act_fn = mybir.ActivationFunctionType

    nc = bacc.Bacc("TRN2", target_bir_lowering=False)
    z = nc.dram_tensor("z", [N_LOC, D], fp32, kind="ExternalInput")
    w = nc.dram_tensor("w", [K, D], fp32, kind="ExternalInput")
    out = nc.dram_tensor("out", [N_LOC, D], fp32, kind="ExternalOutput")
    s2row_dram = nc.dram_tensor("s2row_scratch", [1, K], fp32, kind="Internal")

    with tile.TileContext(nc) as tc:
        with (
            tc.tile_pool(name="const", bufs=1) as constp,
            tc.tile_pool(name="sb", bufs=3) as sb,
            tc.tile_pool(name="big", bufs=2) as big,
            tc.tile_pool(name="ps", bufs=2, space="PSUM") as ps,
            tc.tile_pool(name="psd", bufs=2, space="PSUM") as psd,
        ):
            ident = constp.tile([P, P], fp32, tag="ident")
            make_identity(nc, ident[:])
            onesrow = constp.tile([1, P], fp32, tag="onesrow")
            nc.vector.memset(onesrow[:], 1.0)

            # wT[d, dc*K + k] = w[k, dc*128 + d]
            wT = constp.tile([P, 2 * K], fp32, tag="wT")
            wnats = []
            for c in range(K // P):
                wnat = constp.tile([P, D], fp32, tag=f"wnat{c}")
                wnats.append(wnat)
                nc.sync.dma_start(wnat[:], w[c * P:(c + 1) * P, :])
                for dc in range(2):
                    tp = ps.tile([P, P], fp32, tag="tp")
                    nc.tensor.transpose(
                        tp[:], wnat[:, dc * P:(dc + 1) * P], ident[:]
                    )
                    nc.scalar.copy(
                        wT[:, dc * K + c * P: dc * K + (c + 1) * P], tp[:]
                    )

            # S2_k = sum_d w[k,d]^2 in block8x32 order; broadcast to all
            # partitions via a K=1 ones matmul (exact single products).
            w2col = constp.tile([P, 8], fp32, tag="w2col")
            for c in range(8):
                wsq = sb.tile([P, D], fp32, tag="wsq")
                nc.vector.tensor_tensor(
                    out=wsq[:], in0=wnats[c][:], in1=wnats[c][:], op=alu.mult
                )
                r8w = sb.tile([P, 8], fp32, tag="r8w")
                nc.vector.tensor_reduce(
                    out=r8w[:], in_=wsq[:].rearrange("p (a b) -> p a b", b=32),
                    axis=mybir.AxisListType.X, op=alu.add,
                )
                nc.vector.tensor_reduce(
                    out=w2col[:, c:c + 1], in_=r8w[:],
                    axis=mybir.AxisListType.X, op=alu.add,
                )
            w2T = ps.tile([P, P], fp32, tag="tp")
            nc.tensor.transpose(w2T[:8, :], w2col[:], ident[:])
            w2Tsb = sb.tile([8, P], fp32, tag="w2Tsb")
            nc.scalar.copy(w2Tsb[:], w2T[:8, :])
            nc.sync.dma_start(
                s2row_dram[:].rearrange("o (a b) -> (o a) b", b=P), w2Tsb[:]
            )
            s2row = constp.tile([1, K], fp32, tag="s2row")
            nc.sync.dma_start(s2row[:], s2row_dram[:])
            s2ps = psd.tile([P, K], fp32, tag="dist")
            for h in range(2):
                nc.tensor.matmul(
                    s2ps[:, h * 512:(h + 1) * 512], lhsT=onesrow[:],
                    rhs=s2row[:, h * 512:(h + 1) * 512], start=True, stop=True,
                )
            s2full = constp.tile([P, K], fp32, tag="s2full")
            nc.scalar.copy(s2full[:], s2ps[:])

            for i in range(n_tiles):
                zt = sb.tile([P, D], fp32, tag="zt")
                nc.sync.dma_start(zt[:], z[i * P:(i + 1) * P, :])

                zsq = sb.tile([P, D], fp32, tag="zsq")
                nc.scalar.activation(zsq[:], zt[:], act_fn.Square)
                r8 = sb.tile([P, 8], fp32, tag="r8")
                nc.vector.tensor_reduce(
                    out=r8[:], in_=zsq[:].rearrange("p (a b) -> p a b", b=32),
                    axis=mybir.AxisListType.X, op=alu.add,
                )
                s1 = sb.tile([P, 1], fp32, tag="s1")
                nc.vector.tensor_reduce(
                    out=s1[:], in_=r8[:], axis=mybir.AxisListType.X, op=alu.add
                )

                t1 = big.tile([P, K], fp32, tag="t1")
                nc.scalar.activation(
                    t1[:], s2full[:], act_fn.Identity, bias=s1[:, :1], scale=1.0
                )

                ztp = ps.tile([P, D], fp32, tag="ztp")
                for dc in range(2):
                    nc.tensor.transpose(
                        ztp[:, dc * P:(dc + 1) * P],
                        zt[:, dc * P:(dc + 1) * P], ident[:],
                    )
                zT = sb.tile([P, D], fp32, tag="zT")
                nc.scalar.mul(zT[:], ztp[:], 2.0)

                m2 = psd.tile([P, K], fp32, tag="dist")
                for h in range(2):
                    for dc in range(2):
                        nc.tensor.matmul(
                            m2[:, h * 512:(h + 1) * 512],
                            lhsT=zT[:, dc * P:(dc + 1) * P],
                            rhs=wT[:, dc * K + h * 512: dc * K + (h + 1) * 512],
                            start=(dc == 0), stop=(dc == 1),
                        )

                yv = big.tile([P, K], fp32, tag="yv")
                nc.vector.scalar_tensor_tensor(
                    out=yv[:], in0=m2[:], scalar=1.0, in1=t1[:],
                    op0=alu.mult, op1=alu.subtract,
                )
                mx = sb.tile([P, 8], fp32, tag="mx")
                ix = sb.tile([P, 8], u32, tag="ix")
                nc.vector.max(out=mx[:], in_=yv[:])
                nc.vector.max_index(out=ix[:], in_max=mx[:], in_values=yv[:])

                rows = sb.tile([P, D], fp32, tag="rows")
                nc.gpsimd.indirect_dma_start(
                    out=rows[:], out_offset=None, in_=w[:],
                    in_offset=bass.IndirectOffsetOnAxis(ap=ix[:, :1], axis=0),
                )
                d1 = sb.tile([P, D], fp32, tag="d1")
                nc.gpsimd.tensor_tensor(
                    out=d1[:], in0=rows[:], in1=zt[:], op=alu.subtract
                )
                o = sb.tile([P, D], fp32, tag="o")
                nc.gpsimd.tensor_tensor(
                    out=o[:], in0=zt[:], in1=d1[:], op=alu.add
                )
                nc.sync.dma_start(out[i * P:(i + 1) * P, :], o[:])

    nc.finalize()
    return nc


def _get_nc():
    if "nc" not in _CACHE:
        _CACHE["nc"] = _build()
    return _CACHE["nc"]


def kernel(z_e: np.ndarray, weight: np.ndarray, _trace: bool = False):
    from concourse.bass_utils import run_bass_kernel_spmd

    z_flat = np.ascontiguousarray(
        np.asarray(z_e, dtype=np.float32).reshape(N_TOTAL, D)
    )
    w_np = np.ascontiguousarray(np.asarray(weight, dtype=np.float32))
    in_maps = [
        {"z": z_flat[c * N_LOC:(c + 1) * N_LOC], "w": w_np}
        for c in range(N_CORES)
    ]
    nc = _get_nc()
    res = run_bass_kernel_spmd(
        nc, in_maps, core_ids=list(range(N_CORES)), trace=_trace
    )
    z_q = np.concatenate([res.results[c]["out"] for c in range(N_CORES)], axis=0)
    z_q = z_q.reshape(np.asarray(z_e).shape)
    if _trace:
        kernel.last_result = res
    return z_q
